# revision 4
# baseline (speedup 1.0000x reference)
"""LinOSS layer Trainium2 kernel (pipelined, merged r/i layout).

Math (same derivation as before): per-state recurrence collapses to
rank-2 modulated prefix sums
    u_t = s * Bu_t
    E = cumsum(T1 * u);  F = cumsum(T2 * u)
    x_t = sin(t th) * (E_t + oE) + cos(t th) * (F_t + oF)
    T1 = gamma*cos + sin;  T2 = cos - gamma*sin
with oE/oF the fold-chunk carry offsets.

Layout: 128 partitions = 2 fold-chunks x (32 real + 32 imag states);
free dim = 4096 time cols.  r/i share theta so one table row set serves
both; E/F merge the four scans of the old layout into two.

Pipeline: host sends input pre-transposed plus all four tables (no
on-chip table build, no DMA transpose).  Per 1024-col chunk: Bu matmuls
-> PSUM, DVE stt modulate straight from PSUM (accum_out collects row
sums for the carry), chained DVE scans (AP initial).  Carry offsets are
ready after the last modulate, so demod (ACT bias-add, DVE/Pool mults)
and projection matmuls overlap the remaining scans.
"""

import numpy as np

L, H, P = 8192, 128, 256
NCORES = 8
SLOC = P // NCORES          # 32 states per core
RI = 2 * SLOC               # 64 r+i rows per fold chunk
FOLD = 2
CL = L // FOLD              # 4096 free cols
SC = 1024                   # modulate/scan chunk
K = CL // SC                # 4
JT = 512                    # demod/project chunk
NJ = CL // JT               # 8

_CACHE: dict = {}


def _build_bass(split_waits=True):
    import concourse.bass as bass
    import concourse.mybir as mybir
    import concourse.tile as tile

    dt = mybir.dt.float32
    bt = mybir.dt.bfloat16
    Alu = mybir.AluOpType
    Ident = mybir.ActivationFunctionType.Identity

    nc = bass.Bass(
        trn_type="TRN2",
        target_bir_lowering=False,
        debug=False,
        num_devices=NCORES,
    )

    inpT_d = nc.dram_tensor("inpT", [H, L], bt, kind="ExternalInput").ap()
    Bt_d = nc.dram_tensor("Bt", [H, RI], bt, kind="ExternalInput").ap()
    Cx_d = nc.dram_tensor("Cx", [128, H], bt, kind="ExternalInput").ap()
    dD_d = nc.dram_tensor("dD", [H, H], bt, kind="ExternalInput").ap()
    Wm_d = nc.dram_tensor("Wm", [128, 128], dt, kind="ExternalInput").ap()
    T1_d = nc.dram_tensor("T1", [128, CL], bt, kind="ExternalInput").ap()
    T2_d = nc.dram_tensor("T2", [128, CL], bt, kind="ExternalInput").ap()
    sinT_d = nc.dram_tensor("sinT", [128, CL], bt, kind="ExternalInput").ap()
    cosT_d = nc.dram_tensor("cosT", [128, CL], bt, kind="ExternalInput").ap()
    outp = nc.dram_tensor("outp", [H, L], dt, kind="ExternalOutput").ap()

    with tile.TileContext(nc) as tc:
        cpool = tc.alloc_tile_pool(name="const", bufs=1)
        big = tc.alloc_tile_pool(name="big", bufs=1)
        work = tc.alloc_tile_pool(name="work", bufs=2)
        pbu = tc.alloc_tile_pool(name="pbu", bufs=2, space="PSUM")
        pout = tc.alloc_tile_pool(name="pout", bufs=2, space="PSUM")
        poff = tc.alloc_tile_pool(name="poff", bufs=1, space="PSUM")

        Bt = cpool.tile([H, RI], bt)
        Cx = cpool.tile([128, H], bt)
        dD = cpool.tile([H, H], bt)
        Wm = cpool.tile([128, 128], dt)
        inpT = big.tile([H, L], bt, tag="inpT")
        T1 = big.tile([128, CL], bt, tag="T1")
        T2 = big.tile([128, CL], bt, tag="T2")
        sinT = big.tile([128, CL], bt, tag="sinT")
        cosT = big.tile([128, CL], bt, tag="cosT")
        E = big.tile([128, CL], bt, tag="E")
        F = big.tile([128, CL], bt, tag="F")

        ones_b = cpool.tile([128, SC], bt)
        zinit = cpool.tile([128, 1], dt)
        acc1 = cpool.tile([128, K], dt)
        acc2 = cpool.tile([128, K], dt)
        fins = cpool.tile([128, 2], dt)
        offs = cpool.tile([128, 2], dt)

        nc.gpsimd.memset(ones_b[:], 1.0)
        nc.gpsimd.memset(zinit[:], 0.0)

        # -- input / table DMAs --
        # k=0 criticals split into 512-col halves across queues so the first
        # Bu matmul + modulate can start ~4us in; later chunks stream behind
        # on the sync (k=1) and gpsimd (k=2,3) queues; sin/cos (phase 2) on
        # the scalar queue.
        nc.sync.dma_start(out=Bt[:], in_=Bt_d)
        for h in range(2):
            hs = slice(h * 512, (h + 1) * 512)
            nc.sync.dma_start(out=inpT[:, hs], in_=inpT_d[:, hs])
            cs1 = slice(CL + h * 512, CL + (h + 1) * 512)
            nc.sync.dma_start(out=inpT[:, cs1], in_=inpT_d[:, cs1])
            nc.sync.dma_start(out=T1[:, hs], in_=T1_d[:, hs])
            nc.sync.dma_start(out=T2[:, hs], in_=T2_d[:, hs])
        nc.sync.dma_start(out=Cx[:], in_=Cx_d)
        nc.sync.dma_start(out=Wm[:], in_=Wm_d)
        for k in range(1, K):
            q = nc.sync if k == 1 else nc.gpsimd
            cs = slice(k * SC, (k + 1) * SC)
            for c in range(FOLD):
                ic = slice(c * CL + k * SC, c * CL + (k + 1) * SC)
                q.dma_start(out=inpT[:, ic], in_=inpT_d[:, ic])
            q.dma_start(out=T1[:, cs], in_=T1_d[:, cs])
            q.dma_start(out=T2[:, cs], in_=T2_d[:, cs])
            if k == 1:
                nc.gpsimd.dma_start(out=dD[:], in_=dD_d)
        for k in range(K):
            cs = slice(k * SC, (k + 1) * SC)
            nc.scalar.dma_start(out=sinT[:, cs], in_=sinT_d[:, cs])
            nc.scalar.dma_start(out=cosT[:, cs], in_=cosT_d[:, cs])

        # -- phase 1: Bu matmuls -> modulate (stt from PSUM) -> chained scans
        for k in range(K):
            cs = slice(k * SC, (k + 1) * SC)
            pb = pbu.tile([128, SC], dt, tag="bu")
            for h in range(SC // 512):
                hs = slice(h * 512, (h + 1) * 512)
                for c in range(FOLD):
                    mc = c * CL + k * SC + h * 512
                    nc.tensor.matmul(
                        pb[c * RI : (c + 1) * RI, hs],
                        Bt[:], inpT[:, mc : mc + 512],
                        start=True, stop=True,
                        tile_position=(0, c * RI),
                    )
            Y1 = work.tile([128, SC], bt, tag="Y1")
            Y2 = work.tile([128, SC], bt, tag="Y2")
            nc.vector.scalar_tensor_tensor(
                Y1[:], pb[:], 1.0, T1[:, cs], Alu.mult, Alu.mult,
                accum_out=acc1[:, k : k + 1],
            )
            nc.vector.scalar_tensor_tensor(
                Y2[:], pb[:], 1.0, T2[:, cs], Alu.mult, Alu.mult,
                accum_out=acc2[:, k : k + 1],
            )
            if k == K - 1:
                # offsets depend only on the modulate accums: hoist the
                # reduces ahead of the last scans so Wm/offs (PE+ACT) run
                # while DVE finishes scanning.
                nc.vector.tensor_reduce(
                    fins[:, 0:1], acc1[:], mybir.AxisListType.X, Alu.add)
                nc.vector.tensor_reduce(
                    fins[:, 1:2], acc2[:], mybir.AxisListType.X, Alu.add)
            initE = zinit[:] if k == 0 else E[:, k * SC - 1 : k * SC]
            initF = zinit[:] if k == 0 else F[:, k * SC - 1 : k * SC]
            nc.vector.tensor_tensor_scan(
                E[:, cs], ones_b[:], Y1[:], initE, Alu.mult, Alu.add)
            nc.vector.tensor_tensor_scan(
                F[:, cs], ones_b[:], Y2[:], initF, Alu.mult, Alu.add)

        po = poff.tile([128, 2], dt, tag="off")
        nc.tensor.matmul(po[:], Wm[:], fins[:], start=True, stop=True)
        nc.scalar.copy(offs[:], po[:])

        # -- phase 2: demod (bias folded into DVE stts) + project + store --
        for j in range(NJ):
            jc = j * JT
            js = slice(jc, jc + JT)
            m1 = work.tile([128, JT], bt, tag="m1")
            m2 = work.tile([128, JT], bt, tag="m2")
            x0 = work.tile([128, JT], bt, tag="x0")
            nc.vector.scalar_tensor_tensor(
                m1[:], E[:, js], offs[:, 0:1], sinT[:, js], Alu.add, Alu.mult)
            nc.vector.scalar_tensor_tensor(
                m2[:], F[:, js], offs[:, 1:2], cosT[:, js], Alu.add, Alu.mult)
            nc.vector.tensor_add(x0[:], m1[:], m2[:])
            for c in range(FOLD):
                pc = pout.tile([128, JT], dt, tag="o")
                nc.tensor.matmul(
                    pc[:], Cx[c * RI : (c + 1) * RI, :],
                    x0[c * RI : (c + 1) * RI, :],
                    start=True, stop=False,
                    tile_position=(c * RI, 0),
                )
                nc.tensor.matmul(
                    pc[:], dD[:], inpT[:, c * CL + jc : c * CL + jc + JT],
                    start=False, stop=True,
                )
                osb = work.tile([128, JT], dt, tag="osb")
                nc.scalar.copy(osb[:], pc[:])
                nc.sync.dma_start(
                    out=outp[:, c * CL + jc : c * CL + jc + JT], in_=osb[:])

        for p in (poff, pout, pbu, work, big, cpool):
            p.release()
    if split_waits:
        _split_matmul_waits(nc, mybir)
    return nc


def _split_matmul_waits(nc, mybir):
    """Hardware instruction structs fit a limited number of embedded sync
    waits; move extra waits onto an inserted same-queue no-op."""
    caps = {"InstMatmult": 1}
    skip = {"InstNoOp", "InstAllEngineBarrier", "InstSync"}
    k = 0
    for bb in nc.main_func.blocks:
        insts = bb.instructions
        i = 0
        while i < len(insts):
            ins = insts[i]
            tn = type(ins).__name__
            if tn not in skip and ins.sync_info is not None:
                cap = caps.get(tn, 1)
                w = list(ins.sync_info.on_wait or [])
                if len(w) > cap:
                    for wj in w[:-cap]:
                        nop = mybir.InstNoOp(
                            name=f"I-mmdep-{k}",
                            engine=ins.engine,
                            ins=[],
                            outs=[],
                            sync_info=mybir.SyncInfo(
                                on_wait=[wj], on_update=[]
                            ),
                        )
                        k += 1
                        insts.insert(i, nop)
                        i += 1
                    ins.sync_info = mybir.SyncInfo(
                        on_wait=w[-cap:], on_update=ins.sync_info.on_update
                    )
            i += 1


def _host_prep(inputs):
    import ml_dtypes
    f32 = np.float32
    bf16 = ml_dtypes.bfloat16

    inpT = np.ascontiguousarray(
        np.asarray(inputs["input_sequence"], np.float32).T
    ).astype(bf16)
    A = np.maximum(np.asarray(inputs["A_diag_raw"], np.float64), 0.0)
    s = 1.0 / (1.0 + np.exp(-np.asarray(inputs["steps_raw"], np.float64)))
    Br = np.asarray(inputs["B_real"], np.float64)
    Bi = np.asarray(inputs["B_img"], np.float64)
    Cr = np.asarray(inputs["C_real"], np.float64)
    Ci = np.asarray(inputs["C_img"], np.float64)
    D = np.asarray(inputs["D"], np.float64)

    costh = 1.0 - s * s * A / 2.0
    sinth = np.sqrt(np.maximum(1.0 - costh * costh, 1e-300))
    theta = np.arctan2(sinth, costh)
    gamma = (s - s * s * A / 2.0) / sinth

    twopi = 2.0 * np.pi
    j = np.arange(CL, dtype=np.float64)
    in_maps = []
    for kcore in range(NCORES):
        sl = slice(kcore * SLOC, (kcore + 1) * SLOC)
        th_m = np.concatenate([theta[sl], theta[sl]])       # (RI,)
        g_m = np.concatenate([gamma[sl], gamma[sl]])        # (RI,)
        # partitions p = c*RI + m,  absolute time = c*CL + j
        ang = np.empty((128, CL), np.float64)
        for c in range(FOLD):
            tt = (c * CL + j)[None, :] * th_m[:, None]
            ang[c * RI : (c + 1) * RI] = np.mod(tt, twopi)
        sinT = np.sin(ang)
        cosT = np.cos(ang)
        g2 = np.tile(g_m, FOLD)[:, None]
        T1 = g2 * cosT + sinT
        T2 = cosT - g2 * sinT

        Bt = np.empty((H, RI), np.float64)
        Bt[:, 0:SLOC] = (s[sl, None] * Br[sl]).T
        Bt[:, SLOC:RI] = (s[sl, None] * Bi[sl]).T

        Cblk = np.concatenate([Cr[:, sl].T, -Ci[:, sl].T], axis=0)  # (RI, H)
        Cx = np.tile(Cblk, (FOLD, 1))                               # (128, H)

        dD = np.diag(D) if kcore == 0 else np.zeros((H, H))
        Wm = np.zeros((128, 128), f32)
        Wm[np.arange(RI), np.arange(RI) + RI] = 1.0

        in_maps.append({
            "inpT": inpT,
            "Bt": Bt.astype(bf16),
            "Cx": Cx.astype(bf16),
            "dD": dD.astype(bf16),
            "Wm": Wm,
            "T1": T1.astype(bf16),
            "T2": T2.astype(bf16),
            "sinT": sinT.astype(bf16),
            "cosT": cosT.astype(bf16),
        })
    return in_maps


LAST_RESULTS = None


def kernel(**inputs) -> np.ndarray:
    global LAST_RESULTS
    from concourse.bass_utils import run_bass_kernel_spmd

    if "nc" not in _CACHE:
        _CACHE["nc"] = _build_bass()
    nc = _CACHE["nc"]

    in_maps = _host_prep(inputs)
    res = run_bass_kernel_spmd(nc, in_maps, core_ids=list(range(NCORES)))
    LAST_RESULTS = res
    part = np.zeros((H, L), np.float32)
    for r in res.results:
        part += r["outp"]
    return np.ascontiguousarray(part.T)


# revision 8
# speedup vs baseline: 1.2200x; 1.2200x over previous
"""LinOSS layer Trainium2 kernel (pipelined, merged r/i layout).

Math (same derivation as before): per-state recurrence collapses to
rank-2 modulated prefix sums
    u_t = s * Bu_t
    E = cumsum(T1 * u);  F = cumsum(T2 * u)
    x_t = sin(t th) * (E_t + oE) + cos(t th) * (F_t + oF)
    T1 = gamma*cos + sin;  T2 = cos - gamma*sin
with oE/oF the fold-chunk carry offsets.

Layout: 128 partitions = 2 fold-chunks x (32 real + 32 imag states);
free dim = 4096 time cols.  r/i share theta so one table row set serves
both; E/F merge the four scans of the old layout into two.

Pipeline: host sends input pre-transposed plus all four tables (no
on-chip table build, no DMA transpose).  Per 1024-col chunk: Bu matmuls
-> PSUM, DVE stt modulate straight from PSUM (accum_out collects row
sums for the carry), chained DVE scans (AP initial).  Carry offsets are
ready after the last modulate, so demod (ACT bias-add, DVE/Pool mults)
and projection matmuls overlap the remaining scans.
"""

import numpy as np

L, H, P = 8192, 128, 256
NCORES = 8
SLOC = P // NCORES          # 32 states per core
RI = 2 * SLOC               # 64 r+i rows per fold chunk
FOLD = 2
CL = L // FOLD              # 4096 free cols
SC = 1024                   # modulate/scan chunk
K = CL // SC                # 4
JT = 512                    # demod/project chunk
NJ = CL // JT               # 8

_CACHE: dict = {}


def _build_bass(split_waits=True):
    import concourse.bass as bass
    import concourse.mybir as mybir
    import concourse.tile as tile

    dt = mybir.dt.float32
    bt = mybir.dt.bfloat16
    Alu = mybir.AluOpType
    Ident = mybir.ActivationFunctionType.Identity

    nc = bass.Bass(
        trn_type="TRN2",
        target_bir_lowering=False,
        debug=False,
        num_devices=NCORES,
    )

    inpT_d = nc.dram_tensor("inpT", [H, L], bt, kind="ExternalInput").ap()
    Bt_d = nc.dram_tensor("Bt", [H, RI], bt, kind="ExternalInput").ap()
    Cx_d = nc.dram_tensor("Cx", [128, H], bt, kind="ExternalInput").ap()
    dD_d = nc.dram_tensor("dD", [H, H], bt, kind="ExternalInput").ap()
    Wm_d = nc.dram_tensor("Wm", [128, 128], dt, kind="ExternalInput").ap()
    T1_d = nc.dram_tensor("T1", [128, CL], bt, kind="ExternalInput").ap()
    T2_d = nc.dram_tensor("T2", [128, CL], bt, kind="ExternalInput").ap()
    sinT_d = nc.dram_tensor("sinT", [128, CL], bt, kind="ExternalInput").ap()
    cosT_d = nc.dram_tensor("cosT", [128, CL], bt, kind="ExternalInput").ap()
    outp = nc.dram_tensor("outp", [H, L], bt, kind="ExternalOutput").ap()

    with tile.TileContext(nc) as tc:
        cpool = tc.alloc_tile_pool(name="const", bufs=1)
        big = tc.alloc_tile_pool(name="big", bufs=1)
        work = tc.alloc_tile_pool(name="work", bufs=2)
        pbu = tc.alloc_tile_pool(name="pbu", bufs=2, space="PSUM")
        pout = tc.alloc_tile_pool(name="pout", bufs=2, space="PSUM")
        poff = tc.alloc_tile_pool(name="poff", bufs=1, space="PSUM")

        Bt = cpool.tile([H, RI], bt)
        Cx = cpool.tile([128, H], bt)
        dD = cpool.tile([H, H], bt)
        Wm = cpool.tile([128, 128], dt)
        inpT = big.tile([H, L], bt, tag="inpT")
        T1 = big.tile([128, CL], bt, tag="T1")
        T2 = big.tile([128, CL], bt, tag="T2")
        sinT = big.tile([128, CL], bt, tag="sinT")
        cosT = big.tile([128, CL], bt, tag="cosT")
        E = big.tile([128, CL], bt, tag="E")
        F = big.tile([128, CL], bt, tag="F")

        ones_b = cpool.tile([128, SC], bt)
        zinit = cpool.tile([128, 1], dt)
        acc1 = cpool.tile([128, K], dt)
        acc2 = cpool.tile([128, K], dt)
        fins = cpool.tile([128, 2], dt)
        offs = cpool.tile([128, 2], dt)

        nc.gpsimd.memset(ones_b[:], 1.0)
        nc.gpsimd.memset(zinit[:], 0.0)

        # -- input / table DMAs --
        # k=0/k=1 criticals first on sync, k=2/3 behind on gpsimd; sin/cos
        # (only needed in phase 2) issue at the back of the sync queue so
        # their transfers don't compete with the phase-1 critical path.
        nc.sync.dma_start(out=Bt[:], in_=Bt_d)
        for k in range(2):
            cs = slice(k * SC, (k + 1) * SC)
            for c in range(FOLD):
                ic = slice(c * CL + k * SC, c * CL + (k + 1) * SC)
                nc.sync.dma_start(out=inpT[:, ic], in_=inpT_d[:, ic])
            nc.sync.dma_start(out=T1[:, cs], in_=T1_d[:, cs])
            nc.sync.dma_start(out=T2[:, cs], in_=T2_d[:, cs])
            if k == 0:
                nc.sync.dma_start(out=Cx[:], in_=Cx_d)
                nc.sync.dma_start(out=Wm[:], in_=Wm_d)
        for k in range(2, K):
            cs = slice(k * SC, (k + 1) * SC)
            for c in range(FOLD):
                ic = slice(c * CL + k * SC, c * CL + (k + 1) * SC)
                nc.gpsimd.dma_start(out=inpT[:, ic], in_=inpT_d[:, ic])
            nc.gpsimd.dma_start(out=T1[:, cs], in_=T1_d[:, cs])
            nc.gpsimd.dma_start(out=T2[:, cs], in_=T2_d[:, cs])
        nc.gpsimd.dma_start(out=dD[:], in_=dD_d)
        for k in range(K):
            cs = slice(k * SC, (k + 1) * SC)
            nc.sync.dma_start(out=sinT[:, cs], in_=sinT_d[:, cs])
            nc.sync.dma_start(out=cosT[:, cs], in_=cosT_d[:, cs])

        # -- phase 1: Bu matmuls -> modulate (stt from PSUM) -> chained scans
        for k in range(K):
            cs = slice(k * SC, (k + 1) * SC)
            pb = pbu.tile([128, SC], dt, tag="bu")
            for h in range(SC // 512):
                hs = slice(h * 512, (h + 1) * 512)
                for c in range(FOLD):
                    mc = c * CL + k * SC + h * 512
                    nc.tensor.matmul(
                        pb[c * RI : (c + 1) * RI, hs],
                        Bt[:], inpT[:, mc : mc + 512],
                        start=True, stop=True,
                        tile_position=(0, c * RI),
                    )
            Y1 = work.tile([128, SC], bt, tag="Y1")
            Y2 = work.tile([128, SC], bt, tag="Y2")
            nc.vector.scalar_tensor_tensor(
                Y1[:], pb[:], 1.0, T1[:, cs], Alu.mult, Alu.mult,
                accum_out=acc1[:, k : k + 1],
            )
            nc.vector.scalar_tensor_tensor(
                Y2[:], pb[:], 1.0, T2[:, cs], Alu.mult, Alu.mult,
                accum_out=acc2[:, k : k + 1],
            )
            if k == K - 1:
                # offsets depend only on the modulate accums: hoist the
                # reduces ahead of the last scans so Wm/offs (PE+ACT) run
                # while DVE finishes scanning.
                nc.vector.tensor_reduce(
                    fins[:, 0:1], acc1[:], mybir.AxisListType.X, Alu.add)
                nc.vector.tensor_reduce(
                    fins[:, 1:2], acc2[:], mybir.AxisListType.X, Alu.add)
            initE = zinit[:] if k == 0 else E[:, k * SC - 1 : k * SC]
            initF = zinit[:] if k == 0 else F[:, k * SC - 1 : k * SC]
            nc.vector.tensor_tensor_scan(
                E[:, cs], ones_b[:], Y1[:], initE, Alu.mult, Alu.add)
            nc.vector.tensor_tensor_scan(
                F[:, cs], ones_b[:], Y2[:], initF, Alu.mult, Alu.add)

        po = poff.tile([128, 2], dt, tag="off")
        nc.tensor.matmul(po[:], Wm[:], fins[:], start=True, stop=True)
        nc.scalar.copy(offs[:], po[:])

        # -- phase 2: demod (bias folded into DVE stts) + project + store --
        for j in range(NJ):
            jc = j * JT
            js = slice(jc, jc + JT)
            m1 = work.tile([128, JT], bt, tag="m1")
            m2 = work.tile([128, JT], bt, tag="m2")
            x0 = work.tile([128, JT], bt, tag="x0")
            nc.vector.scalar_tensor_tensor(
                m1[:], E[:, js], offs[:, 0:1], sinT[:, js], Alu.add, Alu.mult)
            nc.vector.scalar_tensor_tensor(
                m2[:], F[:, js], offs[:, 1:2], cosT[:, js], Alu.add, Alu.mult)
            nc.vector.tensor_add(x0[:], m1[:], m2[:])
            for c in range(FOLD):
                pc = pout.tile([128, JT], dt, tag="o")
                nc.tensor.matmul(
                    pc[:], Cx[c * RI : (c + 1) * RI, :],
                    x0[c * RI : (c + 1) * RI, :],
                    start=True, stop=False,
                    tile_position=(c * RI, 0),
                )
                nc.tensor.matmul(
                    pc[:], dD[:], inpT[:, c * CL + jc : c * CL + jc + JT],
                    start=False, stop=True,
                )
                osb = work.tile([128, JT], bt, tag="osb")
                nc.scalar.copy(osb[:], pc[:])
                q = nc.sync if c == 0 else nc.scalar
                q.dma_start(
                    out=outp[:, c * CL + jc : c * CL + jc + JT], in_=osb[:])

        for p in (poff, pout, pbu, work, big, cpool):
            p.release()
    if split_waits:
        _split_matmul_waits(nc, mybir)
    return nc


def _split_matmul_waits(nc, mybir):
    """Hardware instruction structs fit a limited number of embedded sync
    waits; move extra waits onto an inserted same-queue no-op."""
    caps = {"InstMatmult": 1}
    skip = {"InstNoOp", "InstAllEngineBarrier", "InstSync"}
    k = 0
    for bb in nc.main_func.blocks:
        insts = bb.instructions
        i = 0
        while i < len(insts):
            ins = insts[i]
            tn = type(ins).__name__
            if tn not in skip and ins.sync_info is not None:
                cap = caps.get(tn, 1)
                w = list(ins.sync_info.on_wait or [])
                if len(w) > cap:
                    for wj in w[:-cap]:
                        nop = mybir.InstNoOp(
                            name=f"I-mmdep-{k}",
                            engine=ins.engine,
                            ins=[],
                            outs=[],
                            sync_info=mybir.SyncInfo(
                                on_wait=[wj], on_update=[]
                            ),
                        )
                        k += 1
                        insts.insert(i, nop)
                        i += 1
                    ins.sync_info = mybir.SyncInfo(
                        on_wait=w[-cap:], on_update=ins.sync_info.on_update
                    )
            i += 1


def _host_prep(inputs):
    import ml_dtypes
    f32 = np.float32
    bf16 = ml_dtypes.bfloat16

    inpT = np.ascontiguousarray(
        np.asarray(inputs["input_sequence"], np.float32).T
    ).astype(bf16)
    A = np.maximum(np.asarray(inputs["A_diag_raw"], np.float64), 0.0)
    s = 1.0 / (1.0 + np.exp(-np.asarray(inputs["steps_raw"], np.float64)))
    Br = np.asarray(inputs["B_real"], np.float64)
    Bi = np.asarray(inputs["B_img"], np.float64)
    Cr = np.asarray(inputs["C_real"], np.float64)
    Ci = np.asarray(inputs["C_img"], np.float64)
    D = np.asarray(inputs["D"], np.float64)

    costh = 1.0 - s * s * A / 2.0
    sinth = np.sqrt(np.maximum(1.0 - costh * costh, 1e-300))
    theta = np.arctan2(sinth, costh)
    gamma = (s - s * s * A / 2.0) / sinth

    twopi = 2.0 * np.pi
    j = np.arange(CL, dtype=np.float64)
    in_maps = []
    for kcore in range(NCORES):
        sl = slice(kcore * SLOC, (kcore + 1) * SLOC)
        th_m = np.concatenate([theta[sl], theta[sl]])       # (RI,)
        g_m = np.concatenate([gamma[sl], gamma[sl]])        # (RI,)
        # partitions p = c*RI + m,  absolute time = c*CL + j
        ang = np.empty((128, CL), np.float64)
        for c in range(FOLD):
            tt = (c * CL + j)[None, :] * th_m[:, None]
            ang[c * RI : (c + 1) * RI] = np.mod(tt, twopi)
        sinT = np.sin(ang)
        cosT = np.cos(ang)
        g2 = np.tile(g_m, FOLD)[:, None]
        T1 = g2 * cosT + sinT
        T2 = cosT - g2 * sinT

        Bt = np.empty((H, RI), np.float64)
        Bt[:, 0:SLOC] = (s[sl, None] * Br[sl]).T
        Bt[:, SLOC:RI] = (s[sl, None] * Bi[sl]).T

        Cblk = np.concatenate([Cr[:, sl].T, -Ci[:, sl].T], axis=0)  # (RI, H)
        Cx = np.tile(Cblk, (FOLD, 1))                               # (128, H)

        dD = np.diag(D) if kcore == 0 else np.zeros((H, H))
        Wm = np.zeros((128, 128), f32)
        Wm[np.arange(RI), np.arange(RI) + RI] = 1.0

        in_maps.append({
            "inpT": inpT,
            "Bt": Bt.astype(bf16),
            "Cx": Cx.astype(bf16),
            "dD": dD.astype(bf16),
            "Wm": Wm,
            "T1": T1.astype(bf16),
            "T2": T2.astype(bf16),
            "sinT": sinT.astype(bf16),
            "cosT": cosT.astype(bf16),
        })
    return in_maps


LAST_RESULTS = None


def kernel(**inputs) -> np.ndarray:
    global LAST_RESULTS
    from concourse.bass_utils import run_bass_kernel_spmd

    if "nc" not in _CACHE:
        _CACHE["nc"] = _build_bass()
    nc = _CACHE["nc"]

    in_maps = _host_prep(inputs)
    res = run_bass_kernel_spmd(nc, in_maps, core_ids=list(range(NCORES)))
    LAST_RESULTS = res
    part = np.zeros((H, L), np.float32)
    for r in res.results:
        part += np.asarray(r["outp"], np.float32)
    return np.ascontiguousarray(part.T)


# revision 12
# speedup vs baseline: 1.2863x; 1.0543x over previous
"""LinOSS layer Trainium2 kernel (pipelined, merged r/i layout).

Math (same derivation as before): per-state recurrence collapses to
rank-2 modulated prefix sums
    u_t = s * Bu_t
    E = cumsum(T1 * u);  F = cumsum(T2 * u)
    x_t = sin(t th) * (E_t + oE) + cos(t th) * (F_t + oF)
    T1 = gamma*cos + sin;  T2 = cos - gamma*sin
with oE/oF the fold-chunk carry offsets.

Layout: 128 partitions = 2 fold-chunks x (32 real + 32 imag states);
free dim = 4096 time cols.  r/i share theta so one table row set serves
both; E/F merge the four scans of the old layout into two.

Pipeline: host sends input pre-transposed plus all four tables (no
on-chip table build, no DMA transpose).  Per 1024-col chunk: Bu matmuls
-> PSUM, DVE stt modulate straight from PSUM (accum_out collects row
sums for the carry), chained DVE scans (AP initial).  Carry offsets are
ready after the last modulate, so demod (ACT bias-add, DVE/Pool mults)
and projection matmuls overlap the remaining scans.
"""

import numpy as np

L, H, P = 8192, 128, 256
NCORES = 8
SLOC = P // NCORES          # 32 states per core
RI = 2 * SLOC               # 64 r+i rows per fold chunk
FOLD = 2
CL = L // FOLD              # 4096 free cols
SC = 1024                   # modulate/scan chunk
K = CL // SC                # 4
JT = 512                    # demod/project chunk
NJ = CL // JT               # 8

_CACHE: dict = {}


def _build_bass(split_waits=True):
    import concourse.bass as bass
    import concourse.mybir as mybir
    import concourse.tile as tile

    dt = mybir.dt.float32
    bt = mybir.dt.bfloat16
    Alu = mybir.AluOpType
    Ident = mybir.ActivationFunctionType.Identity

    nc = bass.Bass(
        trn_type="TRN2",
        target_bir_lowering=False,
        debug=False,
        num_devices=NCORES,
    )

    inpT_d = nc.dram_tensor("inpT", [H, L], bt, kind="ExternalInput").ap()
    Bt_d = nc.dram_tensor("Bt", [H, RI], bt, kind="ExternalInput").ap()
    Cx_d = nc.dram_tensor("Cx", [128, H], bt, kind="ExternalInput").ap()
    dD_d = nc.dram_tensor("dD", [H, H], bt, kind="ExternalInput").ap()
    Wm_d = nc.dram_tensor("Wm", [128, 128], dt, kind="ExternalInput").ap()
    T1_d = nc.dram_tensor("T1", [128, CL], bt, kind="ExternalInput").ap()
    T2_d = nc.dram_tensor("T2", [128, CL], bt, kind="ExternalInput").ap()
    sinT_d = nc.dram_tensor("sinT", [128, CL], bt, kind="ExternalInput").ap()
    cosT_d = nc.dram_tensor("cosT", [128, CL], bt, kind="ExternalInput").ap()
    outp = nc.dram_tensor("outp", [H, L], bt, kind="ExternalOutput").ap()

    with tile.TileContext(nc) as tc:
        cpool = tc.alloc_tile_pool(name="const", bufs=1)
        big = tc.alloc_tile_pool(name="big", bufs=1)
        work = tc.alloc_tile_pool(name="work", bufs=3)
        opool = tc.alloc_tile_pool(name="opool", bufs=4)
        pbu = tc.alloc_tile_pool(name="pbu", bufs=2, space="PSUM")
        pout = tc.alloc_tile_pool(name="pout", bufs=3, space="PSUM")
        poff = tc.alloc_tile_pool(name="poff", bufs=1, space="PSUM")

        Bt = cpool.tile([H, RI], bt)
        Cx = cpool.tile([128, H], bt)
        dD = cpool.tile([H, H], bt)
        Wm = cpool.tile([128, 128], dt)
        inpT = big.tile([H, L], bt, tag="inpT")
        T1 = big.tile([128, CL], bt, tag="T1")
        T2 = big.tile([128, CL], bt, tag="T2")
        sinT = big.tile([128, CL], bt, tag="sinT")
        cosT = big.tile([128, CL], bt, tag="cosT")
        E = big.tile([128, CL], bt, tag="E")
        F = big.tile([128, CL], bt, tag="F")

        ones_b = cpool.tile([128, SC], bt)
        zinit = cpool.tile([128, 1], dt)
        acc1 = cpool.tile([128, K], dt)
        acc2 = cpool.tile([128, K], dt)
        fins = cpool.tile([128, 2], dt)
        offs = cpool.tile([128, 2], dt)

        nc.gpsimd.memset(ones_b[:], 1.0)
        nc.gpsimd.memset(zinit[:], 0.0)

        # -- input / table DMAs --
        # first criticals spread across three queues (queue bring-up and
        # issue are serial per queue); k=2/3 stream behind on gpsimd;
        # sin/cos (phase 2 only) at the back of sync/scalar queues.
        def icol(c, k):
            return slice(c * CL + k * SC, c * CL + (k + 1) * SC)

        nc.sync.dma_start(out=Bt[:], in_=Bt_d)
        nc.scalar.dma_start(out=inpT[:, icol(0, 0)], in_=inpT_d[:, icol(0, 0)])
        nc.gpsimd.dma_start(out=inpT[:, icol(1, 0)], in_=inpT_d[:, icol(1, 0)])
        nc.sync.dma_start(out=T1[:, 0:SC], in_=T1_d[:, 0:SC])
        nc.scalar.dma_start(out=T2[:, 0:SC], in_=T2_d[:, 0:SC])
        nc.gpsimd.dma_start(out=inpT[:, icol(0, 1)], in_=inpT_d[:, icol(0, 1)])
        nc.sync.dma_start(out=inpT[:, icol(1, 1)], in_=inpT_d[:, icol(1, 1)])
        nc.scalar.dma_start(out=T1[:, SC : 2 * SC], in_=T1_d[:, SC : 2 * SC])
        nc.sync.dma_start(out=T2[:, SC : 2 * SC], in_=T2_d[:, SC : 2 * SC])
        nc.scalar.dma_start(out=Cx[:], in_=Cx_d)
        nc.sync.dma_start(out=Wm[:], in_=Wm_d)
        for k in range(2, K):
            cs = slice(k * SC, (k + 1) * SC)
            for c in range(FOLD):
                nc.gpsimd.dma_start(out=inpT[:, icol(c, k)],
                                    in_=inpT_d[:, icol(c, k)])
            nc.gpsimd.dma_start(out=T1[:, cs], in_=T1_d[:, cs])
            nc.gpsimd.dma_start(out=T2[:, cs], in_=T2_d[:, cs])
        nc.gpsimd.dma_start(out=dD[:], in_=dD_d)
        for k in range(K):
            cs = slice(k * SC, (k + 1) * SC)
            nc.sync.dma_start(out=sinT[:, cs], in_=sinT_d[:, cs])
            nc.scalar.dma_start(out=cosT[:, cs], in_=cosT_d[:, cs])

        # -- phase 1: Bu matmuls -> modulate (stt from PSUM) -> chained scans
        for k in range(K):
            cs = slice(k * SC, (k + 1) * SC)
            pb = pbu.tile([128, SC], dt, tag="bu")
            for h in range(SC // 512):
                hs = slice(h * 512, (h + 1) * 512)
                for c in range(FOLD):
                    mc = c * CL + k * SC + h * 512
                    nc.tensor.matmul(
                        pb[c * RI : (c + 1) * RI, hs],
                        Bt[:], inpT[:, mc : mc + 512],
                        start=True, stop=True,
                        tile_position=(0, c * RI),
                    )
            Y1 = work.tile([128, SC], bt, tag="Y1")
            Y2 = work.tile([128, SC], bt, tag="Y2")
            nc.vector.scalar_tensor_tensor(
                Y1[:], pb[:], 1.0, T1[:, cs], Alu.mult, Alu.mult,
                accum_out=acc1[:, k : k + 1],
            )
            nc.vector.scalar_tensor_tensor(
                Y2[:], pb[:], 1.0, T2[:, cs], Alu.mult, Alu.mult,
                accum_out=acc2[:, k : k + 1],
            )
            if k == K - 1:
                # offsets depend only on the modulate accums: hoist the
                # reduces ahead of the last scans so Wm/offs (PE+ACT) run
                # while DVE finishes scanning.
                nc.vector.tensor_reduce(
                    fins[:, 0:1], acc1[:], mybir.AxisListType.X, Alu.add)
                nc.vector.tensor_reduce(
                    fins[:, 1:2], acc2[:], mybir.AxisListType.X, Alu.add)
            initE = zinit[:] if k == 0 else E[:, k * SC - 1 : k * SC]
            initF = zinit[:] if k == 0 else F[:, k * SC - 1 : k * SC]
            nc.vector.tensor_tensor_scan(
                E[:, cs], ones_b[:], Y1[:], initE, Alu.mult, Alu.add)
            nc.vector.tensor_tensor_scan(
                F[:, cs], ones_b[:], Y2[:], initF, Alu.mult, Alu.add)

        po = poff.tile([128, 2], dt, tag="off")
        nc.tensor.matmul(po[:], Wm[:], fins[:], start=True, stop=True)
        nc.scalar.copy(offs[:], po[:])

        # -- phase 2: demod (bias folded into DVE stts) + project + store --
        for j in range(NJ):
            jc = j * JT
            js = slice(jc, jc + JT)
            m1 = work.tile([128, JT], bt, tag="m1")
            m2 = work.tile([128, JT], bt, tag="m2")
            x0 = work.tile([128, JT], bt, tag="x0")
            nc.vector.scalar_tensor_tensor(
                m1[:], E[:, js], offs[:, 0:1], sinT[:, js], Alu.add, Alu.mult)
            nc.vector.scalar_tensor_tensor(
                m2[:], F[:, js], offs[:, 1:2], cosT[:, js], Alu.add, Alu.mult)
            nc.vector.tensor_add(x0[:], m1[:], m2[:])
            for c in range(FOLD):
                pc = pout.tile([128, JT], dt, tag="o")
                nc.tensor.matmul(
                    pc[:], Cx[c * RI : (c + 1) * RI, :],
                    x0[c * RI : (c + 1) * RI, :],
                    start=True, stop=False,
                    tile_position=(c * RI, 0),
                )
                nc.tensor.matmul(
                    pc[:], dD[:], inpT[:, c * CL + jc : c * CL + jc + JT],
                    start=False, stop=True,
                )
                osb = opool.tile([128, JT], bt, tag="osb")
                nc.scalar.copy(osb[:], pc[:])
                q = nc.sync if c == 0 else nc.scalar
                q.dma_start(
                    out=outp[:, c * CL + jc : c * CL + jc + JT], in_=osb[:])

        for p in (poff, pout, pbu, opool, work, big, cpool):
            p.release()
    if split_waits:
        _split_matmul_waits(nc, mybir)
    return nc


def _split_matmul_waits(nc, mybir):
    """Hardware instruction structs fit a limited number of embedded sync
    waits; move extra waits onto an inserted same-queue no-op."""
    caps = {"InstMatmult": 1}
    skip = {"InstNoOp", "InstAllEngineBarrier", "InstSync"}
    k = 0
    for bb in nc.main_func.blocks:
        insts = bb.instructions
        i = 0
        while i < len(insts):
            ins = insts[i]
            tn = type(ins).__name__
            if tn not in skip and ins.sync_info is not None:
                cap = caps.get(tn, 1)
                w = list(ins.sync_info.on_wait or [])
                if len(w) > cap:
                    for wj in w[:-cap]:
                        nop = mybir.InstNoOp(
                            name=f"I-mmdep-{k}",
                            engine=ins.engine,
                            ins=[],
                            outs=[],
                            sync_info=mybir.SyncInfo(
                                on_wait=[wj], on_update=[]
                            ),
                        )
                        k += 1
                        insts.insert(i, nop)
                        i += 1
                    ins.sync_info = mybir.SyncInfo(
                        on_wait=w[-cap:], on_update=ins.sync_info.on_update
                    )
            i += 1


def _host_prep(inputs):
    import ml_dtypes
    f32 = np.float32
    bf16 = ml_dtypes.bfloat16

    inpT = np.ascontiguousarray(
        np.asarray(inputs["input_sequence"], np.float32).T
    ).astype(bf16)
    A = np.maximum(np.asarray(inputs["A_diag_raw"], np.float64), 0.0)
    s = 1.0 / (1.0 + np.exp(-np.asarray(inputs["steps_raw"], np.float64)))
    Br = np.asarray(inputs["B_real"], np.float64)
    Bi = np.asarray(inputs["B_img"], np.float64)
    Cr = np.asarray(inputs["C_real"], np.float64)
    Ci = np.asarray(inputs["C_img"], np.float64)
    D = np.asarray(inputs["D"], np.float64)

    costh = 1.0 - s * s * A / 2.0
    sinth = np.sqrt(np.maximum(1.0 - costh * costh, 1e-300))
    theta = np.arctan2(sinth, costh)
    gamma = (s - s * s * A / 2.0) / sinth

    twopi = 2.0 * np.pi
    j = np.arange(CL, dtype=np.float64)
    in_maps = []
    for kcore in range(NCORES):
        sl = slice(kcore * SLOC, (kcore + 1) * SLOC)
        th_m = np.concatenate([theta[sl], theta[sl]])       # (RI,)
        g_m = np.concatenate([gamma[sl], gamma[sl]])        # (RI,)
        # partitions p = c*RI + m,  absolute time = c*CL + j
        ang = np.empty((128, CL), np.float64)
        for c in range(FOLD):
            tt = (c * CL + j)[None, :] * th_m[:, None]
            ang[c * RI : (c + 1) * RI] = np.mod(tt, twopi)
        sinT = np.sin(ang)
        cosT = np.cos(ang)
        g2 = np.tile(g_m, FOLD)[:, None]
        T1 = g2 * cosT + sinT
        T2 = cosT - g2 * sinT

        Bt = np.empty((H, RI), np.float64)
        Bt[:, 0:SLOC] = (s[sl, None] * Br[sl]).T
        Bt[:, SLOC:RI] = (s[sl, None] * Bi[sl]).T

        Cblk = np.concatenate([Cr[:, sl].T, -Ci[:, sl].T], axis=0)  # (RI, H)
        Cx = np.tile(Cblk, (FOLD, 1))                               # (128, H)

        dD = np.diag(D) if kcore == 0 else np.zeros((H, H))
        Wm = np.zeros((128, 128), f32)
        Wm[np.arange(RI), np.arange(RI) + RI] = 1.0

        in_maps.append({
            "inpT": inpT,
            "Bt": Bt.astype(bf16),
            "Cx": Cx.astype(bf16),
            "dD": dD.astype(bf16),
            "Wm": Wm,
            "T1": T1.astype(bf16),
            "T2": T2.astype(bf16),
            "sinT": sinT.astype(bf16),
            "cosT": cosT.astype(bf16),
        })
    return in_maps


LAST_RESULTS = None


def kernel(**inputs) -> np.ndarray:
    global LAST_RESULTS
    from concourse.bass_utils import run_bass_kernel_spmd

    if "nc" not in _CACHE:
        _CACHE["nc"] = _build_bass()
    nc = _CACHE["nc"]

    in_maps = _host_prep(inputs)
    res = run_bass_kernel_spmd(nc, in_maps, core_ids=list(range(NCORES)))
    LAST_RESULTS = res
    part = np.zeros((H, L), np.float32)
    for r in res.results:
        part += np.asarray(r["outp"], np.float32)
    return np.ascontiguousarray(part.T)
